# revision 4
# baseline (speedup 1.0000x reference)
"""Bahdanau attention Trainium2 kernel (8 NeuronCores, batch-parallel).

reference:
    enc_p = einsum("bsd,du->bsu", encoder_output, W1) + b1
    dec_p = (decoder_state @ W2 + b2)[:, None, :]
    h     = tanh(enc_p + dec_p)
    score = einsum("bsu,uo->bso", h, V) + bv
    att   = softmax(score, axis=1)
    ctx   = sum(att * encoder_output, axis=1)
    returns (ctx [B,D], att [B,S,1])

Strategy: shard batch over the 8 cores (4 batches/core, no collectives).
Single pass over encoder_output per core, bf16 compute, fp32 accumulate:
  per 512-row s-block:
    E [128,4*512] f32 -> bf16 cast -> PE-transpose -> ET[d,s] (FWL bf16)
    hT[u,s] = W1.T @ ET  (PSUM f32), tanh+bias(dec_p) on ScalarE -> bf16
    scoreT[s,1] = hT.T @ V
    w = exp(score+bv)  (ScalarE; accum_out accumulates softmax denom)
    ctx_psum[1,512] += w.T @ E16  (one PSUM group across the whole batch)
  epilogue per batch: l = sum(w); ctx *= 1/l; att = w * (1/l), stored
  via a PE transpose so the DMA out is contiguous.
"""

import sys

sys.path.insert(0, "/opt/trn_rl_repo")

import numpy as np

N_CORES = 8
B, S, D, U = 32, 4096, 512, 512
NB = B // N_CORES  # batches per core

_COMPILED = {}
TRACE = False
LAST_RESULT = None


def _build(nb, s_len):
    import concourse.bacc as bacc
    import concourse.mybir as mybir
    import concourse.tile as tile

    f32 = mybir.dt.float32
    bf16 = mybir.dt.bfloat16
    AF = mybir.ActivationFunctionType

    nk = s_len // 512  # s-blocks per batch
    nj = s_len // 128  # s-tiles (128 rows) per batch

    nc = bacc.Bacc("TRN2", target_bir_lowering=False, debug=False,
                   num_devices=N_CORES)

    enc = nc.declare_dram_parameter("enc", [nb, s_len, D], f32, isOutput=False)
    dec_t = nc.declare_dram_parameter("dec_t", [128, 4 * nb], bf16, isOutput=False)
    w1 = nc.declare_dram_parameter("w1", [128, 4 * U], bf16, isOutput=False)
    w2 = nc.declare_dram_parameter("w2", [128, 4 * U], bf16, isOutput=False)
    b12 = nc.declare_dram_parameter("b12", [128, 4], f32, isOutput=False)
    vt = nc.declare_dram_parameter("vt", [128, 4], bf16, isOutput=False)
    bvr = nc.declare_dram_parameter("bvr", [128, 1], f32, isOutput=False)
    onesc = nc.declare_dram_parameter("onesc", [128, 1], f32, isOutput=False)
    onesr = nc.declare_dram_parameter("onesr", [1, 128], f32, isOutput=False)
    ident = nc.declare_dram_parameter("ident", [128, 128], bf16, isOutput=False)
    ctx_out = nc.declare_dram_parameter("ctx_out", [nb, D], f32, isOutput=True)
    att_out = nc.declare_dram_parameter("att_out", [nb, s_len], f32, isOutput=True)

    enc_ap = enc.ap()
    # att viewed as [b, j, p] with s = j*128 + p
    att3 = att_out.ap().rearrange("b (j p) -> b j p", p=128)
    ctx_ap = ctx_out.ap()

    with tile.TileContext(nc) as tc:
        with (
            tc.tile_pool(name="const", bufs=1) as cp,
            tc.tile_pool(name="epool", bufs=3) as ep,
            tc.tile_pool(name="e16pool", bufs=3) as e16p,
            tc.tile_pool(name="etpool", bufs=8) as etsb,
            tc.tile_pool(name="htpool", bufs=8) as htp,
            tc.tile_pool(name="wapool", bufs=2) as wap,
            tc.tile_pool(name="finpool", bufs=2) as fp,
            tc.tile_pool(name="etpsum", bufs=2, space="PSUM") as etp,
            tc.tile_pool(name="hpsum", bufs=3, space="PSUM") as hps,
            tc.tile_pool(name="smpsum", bufs=2, space="PSUM") as smp,
            tc.tile_pool(name="ctxpsum", bufs=1, space="PSUM") as cxp,
        ):
            # ---- constants ----
            w1_sb = cp.tile([128, 4 * U], bf16, tag="w1")
            nc.sync.dma_start(out=w1_sb[:], in_=w1.ap()[:])
            w2_sb = cp.tile([128, 4 * U], bf16, tag="w2")
            nc.sync.dma_start(out=w2_sb[:], in_=w2.ap()[:])
            dec_sb = cp.tile([128, 4 * nb], bf16, tag="dec")
            nc.sync.dma_start(out=dec_sb[:], in_=dec_t.ap()[:])
            b12_sb = cp.tile([128, 4], f32, tag="b12")
            nc.sync.dma_start(out=b12_sb[:], in_=b12.ap()[:])
            v_sb = cp.tile([128, 4], bf16, tag="v")
            nc.sync.dma_start(out=v_sb[:], in_=vt.ap()[:])
            bvr_sb = cp.tile([128, 1], f32, tag="bvr")
            nc.sync.dma_start(out=bvr_sb[:], in_=bvr.ap()[:])
            ones_sb = cp.tile([128, 1], f32, tag="ones")
            nc.sync.dma_start(out=ones_sb[:], in_=onesc.ap()[:])
            onesr_sb = cp.tile([1, 128], f32, tag="onesr")
            nc.sync.dma_start(out=onesr_sb[:], in_=onesr.ap()[:])
            ident_sb = cp.tile([128, 128], bf16, tag="ident")
            nc.sync.dma_start(out=ident_sb[:], in_=ident.ap()[:])

            for b in range(nb):
                # ---- decoder projection: bias[u] = W2.T@dec + b1 + b2 ----
                dp = smp.tile([128, 4], f32, tag="smp")
                for uc in range(4):
                    for dc in range(4):
                        nc.tensor.matmul(
                            dp[:, uc:uc + 1],
                            lhsT=w2_sb[:, dc * U + uc * 128: dc * U + (uc + 1) * 128],
                            rhs=dec_sb[:, b * 4 + dc: b * 4 + dc + 1],
                            start=(dc == 0), stop=(dc == 3),
                        )
                bias_b = wap.tile([128, 4], f32, tag="bias")
                for uc in range(4):
                    nc.scalar.activation(
                        bias_b[:, uc:uc + 1], dp[:, uc:uc + 1],
                        AF.Identity, bias=b12_sb[:, uc:uc + 1],
                    )

                watt_b = wap.tile([128, nj], bf16, tag="watt")
                lall_b = wap.tile([128, nk], f32, tag="lall")
                ctxp_b = cxp.tile([1, D], f32, tag="ctxp")

                for k in range(nk):
                    # ---- load E block [512, 512] as one [128, 4*512] tile ----
                    e_t = ep.tile([128, 4 * D], f32, tag="e")
                    nc.sync.dma_start(
                        out=e_t[:].rearrange("p (c d) -> p c d", d=D),
                        in_=enc_ap[b, k * 512:(k + 1) * 512, :]
                        .rearrange("(c p) d -> p c d", p=128),
                    )
                    e16 = e16p.tile([128, 4 * D], bf16, tag="e16")
                    nc.scalar.activation(e16[:], e_t[:], AF.Copy)

                    # ---- transpose E16 -> ET [d, s] ----
                    ets = []
                    for dc in range(4):
                        et_ps = etp.tile([128, 512], bf16, tag="etp")
                        for c in range(4):
                            nc.tensor.transpose(
                                et_ps[:, c * 128:(c + 1) * 128],
                                e16[:, c * D + dc * 128: c * D + (dc + 1) * 128],
                                ident_sb[:],
                            )
                        et_sb = etsb.tile([128, 512], bf16, tag="et")
                        nc.vector.tensor_copy(et_sb[:], et_ps[:])
                        ets.append(et_sb)

                    # ---- hT[u,s] = tanh(W1.T @ ET + bias) ----
                    hts = []
                    for uc in range(4):
                        h_ps = hps.tile([128, 512], f32, tag="hp")
                        for dc in range(4):
                            nc.tensor.matmul(
                                h_ps[:],
                                lhsT=w1_sb[:, dc * U + uc * 128: dc * U + (uc + 1) * 128],
                                rhs=ets[dc][:],
                                start=(dc == 0), stop=(dc == 3),
                            )
                        ht = htp.tile([128, 512], bf16, tag="ht")
                        nc.scalar.activation(
                            ht[:], h_ps[:], AF.Tanh, bias=bias_b[:, uc:uc + 1],
                        )
                        hts.append(ht)

                    # ---- scoreT[s,1] = hT.T @ V ----
                    sc_ps = smp.tile([128, 4], f32, tag="smp")
                    for c in range(4):
                        for uc in range(4):
                            nc.tensor.matmul(
                                sc_ps[:, c:c + 1],
                                lhsT=hts[uc][:, c * 128:(c + 1) * 128],
                                rhs=v_sb[:, uc:uc + 1],
                                start=(uc == 0), stop=(uc == 3),
                            )

                    # ---- w = exp(score + bv); accumulate denom ----
                    nc.scalar.activation(
                        watt_b[:, k * 4:(k + 1) * 4], sc_ps[:],
                        AF.Exp, bias=bvr_sb[:, 0:1],
                        accum_out=lall_b[:, k:k + 1],
                    )

                    # ---- ctx[1,512] += w.T @ E16 ----
                    for c in range(4):
                        j = k * 4 + c
                        nc.tensor.matmul(
                            ctxp_b[:],
                            lhsT=watt_b[:, j:j + 1],
                            rhs=e16[:, c * D:(c + 1) * D],
                            start=(j == 0), stop=(j == nj - 1),
                        )

                # ---- batch epilogue: softmax denom + outputs ----
                lsum = fp.tile([128, 1], f32, tag="lsum")
                nc.vector.tensor_reduce(
                    lsum[:], lall_b[:], axis=mybir.AxisListType.X,
                    op=mybir.AluOpType.add,
                )
                l_ps = smp.tile([1, 1], f32, tag="smp")
                nc.tensor.matmul(
                    l_ps[:], lhsT=ones_sb[:], rhs=lsum[:], start=True, stop=True,
                )
                linv = fp.tile([1, 1], f32, tag="linv")
                nc.vector.reciprocal(linv[:], l_ps[:])
                lr_ps = smp.tile([128, 1], f32, tag="smp")
                nc.tensor.matmul(
                    lr_ps[:], lhsT=onesr_sb[:], rhs=linv[:], start=True, stop=True,
                )
                linv_rep = fp.tile([128, 1], f32, tag="linvrep")
                nc.scalar.copy(linv_rep[:], lr_ps[:])

                # attention out: transpose watt [128, nj] -> [nj, 128], scale
                for q in range(0, nj, 128):
                    qn = min(128, nj - q)
                    wt_ps = smp.tile([128, 128], bf16, tag="smp")
                    nc.tensor.transpose(
                        wt_ps[:qn, :], watt_b[:, q:q + qn], ident_sb[:],
                    )
                    att_sb = fp.tile([128, 128], f32, tag="attsb")
                    nc.scalar.activation(
                        att_sb[:qn, :], wt_ps[:qn, :], AF.Copy,
                        scale=linv_rep[0:qn, 0:1],
                    )
                    nc.sync.dma_start(
                        out=att3[b, q:q + qn, :], in_=att_sb[:qn, :],
                    )

                ctx_sb = fp.tile([1, D], f32, tag="ctxsb")
                nc.scalar.activation(
                    ctx_sb[:], ctxp_b[:], AF.Copy, scale=linv[:],
                )
                nc.sync.dma_start(out=ctx_ap[b:b + 1, :], in_=ctx_sb[:])

    nc.compile()
    return nc


def _get_compiled(nb, s_len):
    key = (nb, s_len)
    if key not in _COMPILED:
        _COMPILED[key] = _build(nb, s_len)
    return _COMPILED[key]


def _prep_shared(W1, b1, W2, b2, V, bv):
    import ml_dtypes

    bf = ml_dtypes.bfloat16
    w1r = np.ascontiguousarray(
        W1.reshape(4, 128, U).transpose(1, 0, 2).reshape(128, 4 * U)).astype(bf)
    w2r = np.ascontiguousarray(
        W2.reshape(4, 128, U).transpose(1, 0, 2).reshape(128, 4 * U)).astype(bf)
    b12 = np.ascontiguousarray((b1 + b2).reshape(4, 128).T)
    vtt = np.ascontiguousarray(V.reshape(4, 128).T).astype(bf)
    bvr = np.full((128, 1), float(np.asarray(bv).reshape(-1)[0]), np.float32)
    return {
        "w1": w1r, "w2": w2r, "b12": b12, "vt": vtt, "bvr": bvr,
        "onesc": np.ones((128, 1), np.float32),
        "onesr": np.ones((1, 128), np.float32),
        "ident": np.eye(128).astype(bf),
    }


def run_sharded(encoder_output, decoder_state, W1, b1, W2, b2, V, bv,
                nb=NB, s_len=S, n_cores=N_CORES, trace=False):
    from concourse.bass_utils import run_bass_kernel_spmd
    import ml_dtypes

    global LAST_RESULT
    enc = np.ascontiguousarray(np.asarray(encoder_output, np.float32))
    dec = np.ascontiguousarray(np.asarray(decoder_state, np.float32))
    shared = _prep_shared(
        np.asarray(W1, np.float32), np.asarray(b1, np.float32),
        np.asarray(W2, np.float32), np.asarray(b2, np.float32),
        np.asarray(V, np.float32).reshape(-1), np.asarray(bv, np.float32))

    nc = _get_compiled(nb, s_len)
    in_maps = []
    for c in range(n_cores):
        enc_sh = enc[c * nb:(c + 1) * nb]
        dec_sh = dec[c * nb:(c + 1) * nb]
        dec_t = np.ascontiguousarray(
            dec_sh.reshape(nb, 4, 128).transpose(2, 0, 1).reshape(128, nb * 4)
        ).astype(ml_dtypes.bfloat16)
        in_maps.append({"enc": enc_sh, "dec_t": dec_t, **shared})

    res = run_bass_kernel_spmd(nc, in_maps, core_ids=list(range(n_cores)),
                               trace=trace)
    LAST_RESULT = res
    ctx = np.concatenate([res.results[i]["ctx_out"] for i in range(n_cores)], 0)
    att = np.concatenate([res.results[i]["att_out"] for i in range(n_cores)], 0)
    return ctx, att.reshape(att.shape[0], s_len, 1)


def kernel(encoder_output, decoder_state, W1, b1, W2, b2, V, bv):
    return run_sharded(encoder_output, decoder_state, W1, b1, W2, b2, V, bv,
                       trace=TRACE)
